# revision 23
# baseline (speedup 1.0000x reference)
"""DeltaNet Trainium2 kernel (nn_DeltaNet_41961830482331) — S0-split chunked form.

Full module: qkv = x @ w_attn; per-(head,group) standardization (ddof=1);
DeltaNet recurrence  S_t = S_{t-1}(0.99 I - 0.01 k k^T) + k v^T, o_t = S_t q_t;
y = o @ w_proj; out = x + y.

Sharding: 8 cores = 4 batches x 2 head-groups (6 heads each); host sums the two
partial y-projections per batch (w_proj row-split across the pair).

Math (per head, chunk n=128, g=0.99, b'=beta/g):  substituting S_t = g^t Sh_t
gives Sh_p = Sh_{p-1}(I - b' k k^T) + k nu_p^T with nu_p = g^-p v, o = Sh_p qh_p,
qh_p = g^p q, and chunk-end rescale Sh0' = g^n Sh_n.  With Mh_p = b' Sh_{p-1} k_p:
  (I + b' stril(G)) Mh = b' K Sh0^T + b' stril(P) K,  G = K K^T, P = K Vnu^T
Split Mh = Ws Sh0^T + U0 where (S0-INDEPENDENT, solved in parallel per chunk):
  [U0n | Ws] = Minv [ -b' stril(P) K | b' K ],  Minv = (I + b' stril G)^{-1}
  (U0n = -U0).  Then everything is two tiny serial matmuls per chunk:
  O^T   = Sh0 OcT + O0T,   OcT = Qh^T - Ws^T Fq,  O0T = K^T Fv + U0n^T Fq
  StT'  = Mser^T StT + CcTg,  Mser = g^n (I - Ws^T K),
  CcTg  = g^n (Vnu^T K + K^T U0n),  Fq = triu(K Qh^T), Fv = triu(Vnu Qh^T)
Minv is applied with a truncated Neumann series (I - A)(I + A^2 + A^4 + A^6 + A^8),
A = b' stril(G), realized as R2 = r - A r then 4 Horner steps z <- R2 + A^2 z.
"""

import numpy as np

B, T, C = 4, 1024, 768
NH, HS = 12, 64
HPC = NH // 2            # heads per core
GAMMA, BETA = 0.99, 0.01
BP = BETA / GAMMA        # beta'
NC_ = 128                # chunk length n
NCH = T // NC_           # chunks
GN = GAMMA ** NC_        # gamma^n
NSOLVE = 4               # Horner double-steps

_cache: dict = {}


def _build_program(debug_taps=False):
    import concourse.bass as bass
    import concourse.tile as tile
    from concourse import bacc, mybir

    f32 = mybir.dt.float32
    bf16 = mybir.dt.bfloat16
    Alu = mybir.AluOpType
    Act = mybir.ActivationFunctionType

    nc = bacc.Bacc()

    W3 = 3 * HPC * HS    # 1152
    KT_ = C // 128       # 6 contraction tiles for qkv proj
    KP = HPC * HS // 128  # 3 contraction tiles for out proj

    # ---- DRAM parameters (per-core data; SPMD: same names on all cores) ----
    xT = nc.dram_tensor("xT", [C, T], bf16, kind="ExternalInput")          # x[b].T
    wA = nc.dram_tensor("wA", [C, W3], bf16, kind="ExternalInput")
    wP = nc.dram_tensor("wP", [HPC * HS, C], bf16, kind="ExternalInput")
    xres = nc.dram_tensor("xres", [T, C], bf16, kind="ExternalInput")      # x[b] or 0
    gvec = nc.dram_tensor("gvec", [128, 2], f32, kind="ExternalInput")     # g^(p+1), g^-(p+1)
    masks = nc.dram_tensor("masks", [128, 640], bf16, kind="ExternalInput")
    ign = nc.dram_tensor("ign", [128, 64], bf16, kind="ExternalInput")     # stacked g^n*I64
    ident = nc.dram_tensor("ident", [128, 128], bf16, kind="ExternalInput")
    y = nc.dram_tensor("y", [T, C], f32, kind="ExternalOutput")
    dbg = {}
    if debug_taps:
        for nm, shp, dt_ in [
            ("d_qkv", [128, W3], bf16), ("d_kn", [128, 128], bf16),
            ("d_vn", [128, 128], bf16), ("d_qn", [128, 128], bf16),
            ("d_tsb", [128, 384], bf16), ("d_gu", [128, 128], bf16),
            ("d_gl", [128, 128], bf16), ("d_fq", [128, 128], bf16),
            ("d_pu", [128, 128], bf16), ("d_fv", [128, 128], bf16),
            ("d_rz", [128, 384], bf16), ("d_r2", [128, 384], bf16),
            ("d_n2", [128, 384], bf16), ("d_w", [128, 384], bf16),
            ("d_oc", [128, 128], bf16), ("d_o0", [128, 128], bf16),
            ("d_ms", [128, 64], bf16), ("d_cc", [128, 64], bf16),
            ("d_st", [128, 64], bf16), ("d_ot", [128, 128], bf16),
            ("d_mu", [128, 18], f32), ("d_rstd", [128, 18], f32),
            ("d_bns", [128, 108], f32),
        ]:
            dbg[nm] = nc.dram_tensor(nm, shp, dt_, kind="ExternalOutput")

    with tile.TileContext(nc) as tc:
        with (
            tc.tile_pool(name="persist", bufs=1) as persist,
            tc.tile_pool(name="qkvp", bufs=2) as qkvp,
            tc.tile_pool(name="statp", bufs=2) as statp,
            tc.tile_pool(name="natp", bufs=2) as natp,
            tc.tile_pool(name="tp", bufs=2) as tp,
            tc.tile_pool(name="gramp", bufs=2) as gramp,
            tc.tile_pool(name="solp", bufs=2) as solp,
            tc.tile_pool(name="serp", bufs=3) as serp,
            tc.tile_pool(name="stp", bufs=2) as stp,
            tc.tile_pool(name="yp", bufs=2) as yp,
            tc.tile_pool(name="ps1", bufs=2, space="PSUM") as ps1,
            tc.tile_pool(name="ps2", bufs=2, space="PSUM") as ps2,
            tc.tile_pool(name="ps3", bufs=2, space="PSUM") as ps3,
            tc.tile_pool(name="ps4", bufs=2, space="PSUM") as ps4,
        ):
            # ---- persistent operands ----
            xT_sb = persist.tile([128, KT_, T], bf16)
            for k in range(KT_):
                nc.gpsimd.dma_start(
                    out=xT_sb[:, k, :],
                    in_=xT.rearrange("(k p) t -> p k t", p=128)[:, k, :])
            wA_sb = persist.tile([128, KT_, W3], bf16)
            for k in range(KT_):
                nc.gpsimd.dma_start(
                    out=wA_sb[:, k, :],
                    in_=wA.rearrange("(k p) j -> p k j", p=128)[:, k, :])
            wP_sb = persist.tile([128, KP, C], bf16)
            nc.gpsimd.dma_start(out=wP_sb, in_=wP.rearrange("(k p) j -> p k j", p=128))
            gv_sb = persist.tile([128, 2], f32)
            nc.gpsimd.dma_start(out=gv_sb, in_=gvec[:, :])
            mk_sb = persist.tile([128, 640], bf16)
            nc.gpsimd.dma_start(out=mk_sb, in_=masks[:, :])
            ig_sb = persist.tile([128, 64], bf16)
            nc.gpsimd.dma_start(out=ig_sb, in_=ign[:, :])
            id_sb = persist.tile([128, 128], bf16)
            nc.gpsimd.dma_start(out=id_sb, in_=ident[:, :])
            # O^T accumulator for the whole sequence: [ch=384, t=1024]
            outT_sb = persist.tile([128, KP, T], bf16)

            # masks layout: [0:128 striu*b' | 128:256 triu_incl | 256:384 -striu
            #                | 384:512 triu_incl | 512:640 stril*b']
            M_GU = mk_sb[:, 0:128]
            M_B = mk_sb[:, 0:256]      # [Gu | Fq]
            M_A = mk_sb[:, 256:512]    # [Pu | Fv]
            M_GL = mk_sb[:, 512:640]

            st_cur = []
            for p in range(3):
                stt0 = stp.tile([128, 64], bf16, tag=f"st{p}")
                nc.vector.memset(stt0, 0.0)
                st_cur.append(stt0)

            # per-chunk state passed between pipeline stages
            st_qkv: dict = {}
            st_mach: dict = {}

            st_raw: dict = {}

            def stage_qkv_mm(c):
                t0 = c * NC_
                qkv_sb = qkvp.tile([128, W3], bf16, tag="qkv")
                for nblk in range(3):
                    pp = ps1.tile([128, 384], f32, tag="p1")
                    for k in range(KT_):
                        nc.tensor.matmul(
                            pp[:, :],
                            lhsT=xT_sb[:, k, t0:t0 + 128],
                            rhs=wA_sb[:, k, 384 * nblk:384 * nblk + 384],
                            start=(k == 0), stop=(k == KT_ - 1),
                        )
                    if nblk == 2:
                        nc.scalar.copy(qkv_sb[:, 384 * nblk:384 * nblk + 384],
                                       pp[:, :])
                    else:
                        nc.vector.tensor_copy(
                            qkv_sb[:, 384 * nblk:384 * nblk + 384], pp[:, :])
                st_raw[c] = qkv_sb

            def stage_stats(c):
                qkv_sb = st_raw.pop(c)
                qc = qkv_sb[:, :]
                # ---- stats: per 64-col group mean / unbiased rstd ----
                sq = statp.tile([128, W3], bf16, tag="sq")
                nc.vector.tensor_mul(sq[:, :], qc, qc)
                mu = statp.tile([128, 18], f32, tag="mu")
                nc.vector.tensor_reduce(
                    mu[:, :], qc.rearrange("p (g d) -> p g d", d=64),
                    axis=mybir.AxisListType.X, op=Alu.add)
                m2 = statp.tile([128, 18], f32, tag="m2")
                nc.vector.tensor_reduce(
                    m2[:, :], sq.rearrange("p (g d) -> p g d", d=64),
                    axis=mybir.AxisListType.X, op=Alu.add)
                nc.vector.tensor_scalar_mul(mu[:, :], mu[:, :], 1.0 / 64.0)
                nc.vector.tensor_scalar_mul(m2[:, :], m2[:, :], 1.0 / 64.0)
                v63 = statp.tile([128, 18], f32, tag="v63")
                nc.vector.tensor_mul(v63[:, :], mu[:, :], mu[:, :])
                nc.vector.scalar_tensor_tensor(
                    out=v63[:, :], in0=v63[:, :], scalar=-1.0, in1=m2[:, :],
                    op0=Alu.mult, op1=Alu.add)
                rstd = statp.tile([128, 18], f32, tag="rstd")
                nc.scalar.activation(rstd[:, :], v63[:, :], Act.Sqrt,
                                     scale=64.0 / 63.0)
                nc.vector.reciprocal(rstd[:, :], rstd[:, :])
                # rs18: per-group scale with gamma factors folded in
                rs18 = statp.tile([128, 18], f32, tag="rs18")
                nc.vector.tensor_mul(rs18[:, 0:6], rstd[:, 0:6],
                                     gv_sb[:, 0:1].to_broadcast((128, 6)))
                nc.vector.tensor_copy(rs18[:, 6:12], rstd[:, 6:12])
                nc.vector.tensor_mul(rs18[:, 12:18], rstd[:, 12:18],
                                     gv_sb[:, 1:2].to_broadcast((128, 6)))
                # broadcast to full width (Pool), then two wide DVE ops
                mu_b = statp.tile([128, 18, 64], bf16, tag="mu_b")
                nc.gpsimd.tensor_copy(
                    mu_b[:, :, :],
                    mu.rearrange("p (g o) -> p g o", o=1).to_broadcast((128, 18, 64)))
                rs_b = statp.tile([128, 18, 64], bf16, tag="rs_b")
                nc.gpsimd.tensor_copy(
                    rs_b[:, :, :],
                    rs18.rearrange("p (g o) -> p g o", o=1).to_broadcast((128, 18, 64)))
                qn = qkvp.tile([128, W3], bf16, tag="qn")
                nc.vector.tensor_sub(qn[:, :], qc,
                                     mu_b.rearrange("p g d -> p (g d)"))
                nc.vector.tensor_mul(qn[:, :], qn[:, :],
                                     rs_b.rearrange("p g d -> p (g d)"))
                # kb = -b' * khat into rz triads; u0n filled later by mach
                rz_l = []
                for tri in range(2):
                    rz = solp.tile([128, 384], bf16, tag=f"rz{tri}")
                    nc.scalar.mul(
                        rz[:, :].rearrange("p (j f) -> p j f", f=128)[:, :, 64:128],
                        qn[:, 384 + 192 * tri:576 + 192 * tri].rearrange(
                            "p (j f) -> p j f", f=64), -BP)
                    rz_l.append(rz)
                if debug_taps and c == 0:
                    nc.gpsimd.dma_start(out=dbg["d_qkv"][:, :], in_=qkv_sb[:, :])
                    nc.gpsimd.dma_start(out=dbg["d_mu"][:, :], in_=mu[:, :])
                    nc.gpsimd.dma_start(out=dbg["d_rstd"][:, :], in_=rstd[:, :])
                    nc.gpsimd.dma_start(out=dbg["d_kn"][:, :], in_=qn[:, 384:512])
                    nc.gpsimd.dma_start(out=dbg["d_vn"][:, :], in_=qn[:, 768:896])
                    nc.gpsimd.dma_start(out=dbg["d_qn"][:, :], in_=qn[:, 0:128])
                st_qkv[c] = (qn, rz_l)

            def stage_mach(c):
                qn, rz_l = st_qkv.pop(c)
                knp_l = [qn[:, 384 + 128 * p:512 + 128 * p] for p in range(3)]
                vnp_l = [qn[:, 768 + 128 * p:896 + 128 * p] for p in range(3)]
                qnp_l = [qn[:, 128 * p:128 + 128 * p] for p in range(3)]
                # ---- transposes: tsb = [KTpair | QTpair | VTpair] ----
                tsb_l = []
                for p in range(3):
                    tps = ps4.tile([128, 384], bf16, tag="m4")
                    nc.tensor.transpose(tps[:, 0:128], knp_l[p][:, :], id_sb[:, :])
                    nc.tensor.transpose(tps[:, 128:256], qnp_l[p][:, :], id_sb[:, :])
                    nc.tensor.transpose(tps[:, 256:384], vnp_l[p][:, :], id_sb[:, :])
                    tsb = tp.tile([128, 384], bf16, tag=f"tsb{p}")
                    nc.scalar.copy(tsb[:, :], tps[:, :])
                    tsb_l.append(tsb)
                # ---- grams + masked evacs ----
                gu_l, gl_l, fq_l, pu_l, fv_l = [], [], [], [], []
                for i in range(HPC):
                    p, sub = divmod(i, 2)
                    po = 64 * sub
                    kt = tsb_l[p][po:po + 64, 0:128]
                    vt = tsb_l[p][po:po + 64, 256:384]
                    kq = tsb_l[p][po:po + 64, 0:256]     # [KT | QT]
                    gps = ps2.tile([128, 512], f32, tag="gram")
                    nc.tensor.matmul(gps[:, 0:256], lhsT=kt, rhs=kq,
                                     tile_position=(po, 0), skip_group_check=True)
                    nc.tensor.matmul(gps[:, 256:512], lhsT=vt, rhs=kq,
                                     tile_position=(po, 0), skip_group_check=True)
                    gu = gramp.tile([128, 128], bf16, tag=f"gu{i}")
                    nc.vector.tensor_mul(gu[:, :], gps[:, 0:128], M_GU)
                    fqv = gramp.tile([128, 256], bf16, tag=f"fqv{i}")
                    nc.vector.tensor_mul(
                        fqv[:, :].rearrange("p (a f) -> p a f", f=128),
                        gps[:, :].rearrange("p (a b f) -> p a b f", b=2, f=128)[:, :, 1, :],
                        mk_sb[:, 0:512].rearrange("p (a b f) -> p a b f", b=2, f=128)[:, :, 1, :])
                    pu = gramp.tile([128, 128], bf16, tag=f"pu{i}")
                    nc.scalar.copy(pu[:, :], gps[:, 256:384])
                    nc.gpsimd.affine_select(
                        out=pu[:, :], in_=pu[:, :], compare_op=Alu.is_ge,
                        fill=0.0, base=-1, pattern=[[1, 128]],
                        channel_multiplier=-1)
                    glps = ps4.tile([128, 128], bf16, tag="m4")
                    nc.tensor.transpose(glps[:, :], gu[:, :], id_sb[:, :])
                    gl = gramp.tile([128, 128], bf16, tag=f"gl{i}")
                    if i % 2 == 0:
                        nc.vector.tensor_copy(gl[:, :], glps[:, :])
                    else:
                        nc.scalar.copy(gl[:, :], glps[:, :])
                    gu_l.append(gu[:, 0:128]); fq_l.append(fqv[:, 0:128])
                    pu_l.append(pu[:, 0:128]); fv_l.append(fqv[:, 128:256])
                    gl_l.append(gl)
                if debug_taps and c == 0:
                    nc.gpsimd.dma_start(out=dbg["d_tsb"][:, :], in_=tsb_l[0][:, :])
                    nc.gpsimd.dma_start(out=dbg["d_gu"][:, :], in_=gu_l[0])
                    nc.gpsimd.dma_start(out=dbg["d_gl"][:, :], in_=gl_l[0][:, :])
                    nc.gpsimd.dma_start(out=dbg["d_fq"][:, :], in_=fq_l[0])
                    nc.gpsimd.dma_start(out=dbg["d_pu"][:, :], in_=pu_l[0])
                    nc.gpsimd.dma_start(out=dbg["d_fv"][:, :], in_=fv_l[0])

                # ---- u0n = Pu^T kb  (into rz triad cols 128j:128j+64) ----
                for tri in range(2):
                    ups = ps3.tile([128, 192], f32, tag="sol")
                    for j in range(3):
                        i = 3 * tri + j
                        nc.tensor.matmul(ups[:, 64 * j:64 * j + 64],
                                         lhsT=pu_l[i],
                                         rhs=rz_l[tri][:, 128 * j + 64:128 * j + 128],
                                         skip_group_check=True)
                    nc.scalar.copy(
                        rz_l[tri][:, :].rearrange("p (j f) -> p j f", f=128)[:, :, 0:64],
                        ups[:, :].rearrange("p (j f) -> p j f", f=64))
                # ---- n2 = ((b' stril G)^2)^T ;  n4 = n2 @ n2 via n2T ----
                n2_l, n4_l = [], []
                for tri in range(2):
                    nps = ps3.tile([128, 384], f32, tag="sol")
                    for j in range(3):
                        i = 3 * tri + j
                        nc.tensor.matmul(nps[:, 128 * j:128 * j + 128],
                                         lhsT=gl_l[i][:, :], rhs=gu_l[i],
                                         skip_group_check=True)
                    n2 = solp.tile([128, 384], bf16, tag=f"n2{tri}")
                    if tri == 0:
                        nc.vector.tensor_copy(n2[:, :], nps[:, :])
                    else:
                        nc.scalar.copy(n2[:, :], nps[:, :])
                    n2_l.append(n2)
                for tri in range(2):
                    tps2 = ps4.tile([128, 384], bf16, tag="m4")
                    for j in range(3):
                        nc.tensor.transpose(tps2[:, 128 * j:128 * j + 128],
                                            n2_l[tri][:, 128 * j:128 * j + 128],
                                            id_sb[:, :])
                    n2t = solp.tile([128, 384], bf16, tag=f"n2t{tri}")
                    nc.scalar.copy(n2t[:, :], tps2[:, :])
                    nps4 = ps3.tile([128, 384], f32, tag="sol")
                    for j in range(3):
                        nc.tensor.matmul(nps4[:, 128 * j:128 * j + 128],
                                         lhsT=n2t[:, 128 * j:128 * j + 128],
                                         rhs=n2_l[tri][:, 128 * j:128 * j + 128],
                                         skip_group_check=True)
                    n4 = solp.tile([128, 384], bf16, tag=f"n4{tri}")
                    if tri == 0:
                        nc.vector.tensor_copy(n4[:, :], nps4[:, :])
                    else:
                        nc.scalar.copy(n4[:, :], nps4[:, :])
                    n4_l.append(n4)
                # ---- R2 = (I - A) rz ----
                r2_l = []
                for tri in range(2):
                    rps = ps3.tile([128, 384], f32, tag="sol")
                    nc.tensor.matmul(rps[:, :], lhsT=id_sb[:, :],
                                     rhs=rz_l[tri][:, :],
                                     start=True, stop=False,
                                     skip_group_check=True)
                    for j in range(3):
                        i = 3 * tri + j
                        nc.tensor.matmul(rps[:, 128 * j:128 * j + 128],
                                         lhsT=gu_l[i],
                                         rhs=rz_l[tri][:, 128 * j:128 * j + 128],
                                         start=False, stop=(j == 2),
                                         skip_group_check=True)
                    r2 = solp.tile([128, 384], bf16, tag=f"r2{tri}")
                    if tri == 0:
                        nc.vector.tensor_copy(r2[:, :], rps[:, :])
                    else:
                        nc.scalar.copy(r2[:, :], rps[:, :])
                    r2_l.append(r2)
                # ---- z = (I + A^2)(I + A^4) style: s2 = r2 + A^2 r2; W = s2 + A^4 s2
                z_l = [None, None]
                w_l = [None, None]
                for stage, nn_l in ((0, n2_l), (1, n4_l)):
                    for tri in range(2):
                        src_t = r2_l[tri] if stage == 0 else z_l[tri]
                        zps = ps3.tile([128, 384], f32, tag="sol")
                        nc.tensor.matmul(zps[:, :], lhsT=id_sb[:, :],
                                         rhs=src_t[:, :],
                                         start=True, stop=False,
                                         skip_group_check=True)
                        for j in range(3):
                            nc.tensor.matmul(
                                zps[:, 128 * j:128 * j + 128],
                                lhsT=nn_l[tri][:, 128 * j:128 * j + 128],
                                rhs=src_t[:, 128 * j:128 * j + 128],
                                start=False, stop=(j == 2),
                                skip_group_check=True)
                        tag = f"w{tri}" if stage == 1 else f"z{tri}"
                        zn = solp.tile([128, 384], bf16, tag=tag)
                        if tri == 0:
                            nc.vector.tensor_copy(zn[:, :], zps[:, :])
                        else:
                            nc.scalar.copy(zn[:, :], zps[:, :])
                        if stage == 0:
                            z_l[tri] = zn
                        else:
                            w_l[tri] = zn
                if debug_taps and c == 0:
                    nc.gpsimd.dma_start(out=dbg["d_rz"][:, :], in_=rz_l[0][:, :])
                    nc.gpsimd.dma_start(out=dbg["d_r2"][:, :], in_=r2_l[0][:, :])
                    nc.gpsimd.dma_start(out=dbg["d_n2"][:, :], in_=n2_l[0][:, :])
                    nc.gpsimd.dma_start(out=dbg["d_w"][:, :], in_=w_l[0][:, :])

                def wslice(i, blk):  # blk 0 = U0n, 1 = Ws
                    tri, j = divmod(i, 3)
                    return w_l[tri][:, 128 * j + 64 * blk:128 * j + 64 * blk + 64]

                # ---- OcT = Qh^T - Ws^T Fq ; Mser = g^n (I - Ws^T K) ----
                oc_l, ms_l, o0_l, cc_l = [], [], [], []
                for p in range(3):
                    ocps = ps4.tile([128, 192], f32, tag="m4")
                    nc.tensor.matmul(ocps[:, 0:128], lhsT=id_sb[:, :],
                                     rhs=tsb_l[p][:, 128:256],
                                     start=True, stop=False,
                                     skip_group_check=True)
                    for sub in range(2):
                        i = 2 * p + sub
                        po = 64 * sub
                        nc.tensor.matmul(ocps[po:po + 64, 0:128],
                                         lhsT=wslice(i, 1), rhs=fq_l[i],
                                         tile_position=(0, po),
                                         start=False, stop=(sub == 1),
                                         skip_group_check=True)
                        nc.tensor.matmul(ocps[po:po + 64, 128:192],
                                         lhsT=wslice(i, 1),
                                         rhs=knp_l[p][:, po:po + 64],
                                         tile_position=(0, po),
                                         skip_group_check=True)
                    ocP = serp.tile([128, 128], bf16, tag=f"oc{p}")
                    nc.vector.tensor_copy(ocP[:, :], ocps[:, 0:128])
                    msP = serp.tile([128, 64], bf16, tag=f"ms{p}")
                    nc.vector.scalar_tensor_tensor(
                        out=msP[:, :], in0=ocps[:, 128:192], scalar=GN,
                        in1=ig_sb[:, :], op0=Alu.mult, op1=Alu.add)
                    oc_l.append(ocP); ms_l.append(msP)
                # ---- O0T = K^T Fv + U0n^T Fq ; CcTg = g^n (Vn^T K + K^T U0n) ----
                for p in range(3):
                    o0ps = ps4.tile([128, 192], f32, tag="m4")
                    for sub in range(2):
                        i = 2 * p + sub
                        po = 64 * sub
                        nc.tensor.matmul(o0ps[po:po + 64, 0:128],
                                         lhsT=knp_l[p][:, po:po + 64], rhs=fv_l[i],
                                         tile_position=(0, po), start=True,
                                         stop=False, skip_group_check=True)
                        nc.tensor.matmul(o0ps[po:po + 64, 0:128],
                                         lhsT=wslice(i, 0), rhs=fq_l[i],
                                         tile_position=(0, po), start=False,
                                         stop=True, skip_group_check=True)
                        nc.tensor.matmul(o0ps[po:po + 64, 128:192],
                                         lhsT=vnp_l[p][:, po:po + 64],
                                         rhs=knp_l[p][:, po:po + 64],
                                         tile_position=(0, po), start=True,
                                         stop=False, skip_group_check=True)
                        nc.tensor.matmul(o0ps[po:po + 64, 128:192],
                                         lhsT=knp_l[p][:, po:po + 64],
                                         rhs=wslice(i, 0),
                                         tile_position=(0, po), start=False,
                                         stop=True, skip_group_check=True)
                    o0P = serp.tile([128, 128], bf16, tag=f"o0{p}")
                    nc.scalar.copy(o0P[:, :], o0ps[:, 0:128])
                    ccP = serp.tile([128, 64], bf16, tag=f"cc{p}")
                    nc.scalar.mul(ccP[:, :], o0ps[:, 128:192], GN)
                    o0_l.append(o0P); cc_l.append(ccP)
                if debug_taps and c == 0:
                    nc.gpsimd.dma_start(out=dbg["d_oc"][:, :], in_=oc_l[0][:, :])
                    nc.gpsimd.dma_start(out=dbg["d_o0"][:, :], in_=o0_l[0][:, :])
                    nc.gpsimd.dma_start(out=dbg["d_ms"][:, :], in_=ms_l[0][:, :])
                    nc.gpsimd.dma_start(out=dbg["d_cc"][:, :], in_=cc_l[0][:, :])
                st_mach[c] = (oc_l, ms_l, o0_l, cc_l)

            def stage_ser(c):
                t0 = c * NC_
                oc_l, ms_l, o0_l, cc_l = st_mach.pop(c)
                # ---- O^T chunk = Sh0 @ OcT + O0T ----
                for p in range(3):
                    otps = ps4.tile([128, 128], f32, tag="m4")
                    nc.tensor.matmul(otps[:, :], lhsT=id_sb[:, :],
                                     rhs=o0_l[p][:, :],
                                     start=True, stop=False,
                                     skip_group_check=True)
                    for sub in range(2):
                        po = 64 * sub
                        nc.tensor.matmul(otps[po:po + 64, :],
                                         lhsT=st_cur[p][po:po + 64, :],
                                         rhs=oc_l[p][po:po + 64, :],
                                         tile_position=(po, po),
                                         start=False, stop=(sub == 1),
                                         skip_group_check=True)
                    nc.vector.tensor_copy(outT_sb[:, p, t0:t0 + 128], otps[:, :])
                # ---- state: StT' = Mser^T StT + CcTg ----
                for p in range(3):
                    stps = ps4.tile([128, 64], f32, tag="m4")
                    nc.tensor.matmul(stps[:, :], lhsT=id_sb[:, :],
                                     rhs=cc_l[p][:, :],
                                     start=True, stop=False,
                                     skip_group_check=True)
                    for sub in range(2):
                        po = 64 * sub
                        nc.tensor.matmul(stps[po:po + 64, :],
                                         lhsT=ms_l[p][po:po + 64, :],
                                         rhs=st_cur[p][po:po + 64, :],
                                         tile_position=(po, po),
                                         start=False, stop=(sub == 1),
                                         skip_group_check=True)
                    stn = stp.tile([128, 64], bf16, tag=f"st{p}")
                    nc.scalar.copy(stn[:, :], stps[:, :])
                    st_cur[p] = stn
                if debug_taps and c == 0:
                    nc.gpsimd.dma_start(out=dbg["d_st"][:, :], in_=st_cur[0][:, :])
                    nc.gpsimd.dma_start(out=dbg["d_ot"][:, :],
                                        in_=outT_sb[:, 0, t0:t0 + 128])
                # ---- output projection rows t0:t0+128 + residual ----
                xr_sb = yp.tile([128, C], bf16, tag="xr")
                nc.gpsimd.dma_start(out=xr_sb[:, :], in_=xres[t0:t0 + 128, :])
                y_sb = yp.tile([128, C], f32, tag="ysb")
                for nblk in range(2):
                    ypp = ps1.tile([128, 384], f32, tag="p1")
                    nc.tensor.matmul(ypp[:, :], lhsT=id_sb[:, :],
                                     rhs=xr_sb[:, 384 * nblk:384 * nblk + 384],
                                     start=True, stop=False,
                                     skip_group_check=True)
                    for k in range(KP):
                        nc.tensor.matmul(
                            ypp[:, :],
                            lhsT=outT_sb[:, k, t0:t0 + 128],
                            rhs=wP_sb[:, k, 384 * nblk:384 * nblk + 384],
                            start=False, stop=(k == KP - 1),
                            skip_group_check=True,
                        )
                    nc.scalar.copy(y_sb[:, 384 * nblk:384 * nblk + 384],
                                   ypp[:, :])
                nc.gpsimd.dma_start(out=y[t0:t0 + 128, :], in_=y_sb[:, :])

            # ---- software-pipelined emission ----
            stage_qkv_mm(0)
            stage_stats(0)
            stage_qkv_mm(1)
            stage_stats(1)
            stage_mach(0)
            for c in range(2, NCH):
                stage_qkv_mm(c)
                stage_mach(c - 1)
                stage_ser(c - 2)
                stage_stats(c)
            stage_mach(NCH - 1)
            stage_ser(NCH - 2)
            stage_ser(NCH - 1)

    nc.finalize()
    return nc


def _host_inputs(x, w_attn, w_proj):
    """Build the 8 per-core input maps."""
    import ml_dtypes
    bf = ml_dtypes.bfloat16
    in_maps = []
    gvec = np.zeros((128, 2), np.float32)
    p = np.arange(1, 129, dtype=np.float64)
    gvec[:, 0] = GAMMA ** p
    gvec[:, 1] = GAMMA ** (-p)
    striu = np.triu(np.ones((128, 128), np.float32), 1)
    triui = np.triu(np.ones((128, 128), np.float32))
    stril = np.tril(np.ones((128, 128), np.float32), -1)
    masks = np.concatenate(
        [-striu * BP, triui, striu, triui, -stril * BP], axis=1).astype(bf)
    ign = np.concatenate([np.eye(64), np.eye(64)], axis=0).astype(np.float32)
    ign = (ign * GN).astype(bf)
    ident = np.eye(128, dtype=np.float32).astype(bf)
    for core in range(8):
        b, hg = divmod(core, 2)
        h0 = hg * HPC
        cols = []
        for blk in range(3):   # q, k, v column blocks of w_attn
            cols.append(w_attn[:, blk * C + h0 * HS: blk * C + (h0 + HPC) * HS])
        wA_s = np.ascontiguousarray(np.concatenate(cols, axis=1)).astype(bf)
        wP_s = np.ascontiguousarray(w_proj[h0 * HS:(h0 + HPC) * HS]).astype(bf)
        xb = np.ascontiguousarray(x[b])                                # [1024, 768]
        xres = xb.astype(bf) if hg == 0 else np.zeros((T, C), bf)
        in_maps.append({
            "xT": np.ascontiguousarray(xb.T).astype(bf),
            "wA": wA_s,
            "wP": wP_s,
            "xres": xres,
            "gvec": gvec,
            "masks": masks,
            "ign": ign,
            "ident": ident,
        })
    return in_maps


def kernel(x, w_attn, w_proj):
    from concourse.bass_utils import run_bass_kernel_spmd

    if "nc" not in _cache:
        _cache["nc"] = _build_program()
    nc = _cache["nc"]

    in_maps = _host_inputs(np.asarray(x), np.asarray(w_attn), np.asarray(w_proj))
    res = run_bass_kernel_spmd(nc, in_maps, core_ids=list(range(8)))
    out = np.empty((B, T, C), np.float32)
    for b in range(B):
        out[b] = res.results[2 * b]["y"] + res.results[2 * b + 1]["y"]
    return out


# revision 24
# speedup vs baseline: 1.1114x; 1.1114x over previous
"""DeltaNet Trainium2 kernel (nn_DeltaNet_41961830482331) — S0-split chunked form.

Full module: qkv = x @ w_attn; per-(head,group) standardization (ddof=1);
DeltaNet recurrence  S_t = S_{t-1}(0.99 I - 0.01 k k^T) + k v^T, o_t = S_t q_t;
y = o @ w_proj; out = x + y.

Sharding: 8 cores = 4 batches x 2 head-groups (6 heads each); host sums the two
partial y-projections per batch (w_proj row-split across the pair).

Math (per head, chunk n=128, g=0.99, b'=beta/g):  substituting S_t = g^t Sh_t
gives Sh_p = Sh_{p-1}(I - b' k k^T) + k nu_p^T with nu_p = g^-p v, o = Sh_p qh_p,
qh_p = g^p q, and chunk-end rescale Sh0' = g^n Sh_n.  With Mh_p = b' Sh_{p-1} k_p:
  (I + b' stril(G)) Mh = b' K Sh0^T + b' stril(P) K,  G = K K^T, P = K Vnu^T
Split Mh = Ws Sh0^T + U0 where (S0-INDEPENDENT, solved in parallel per chunk):
  [U0n | Ws] = Minv [ -b' stril(P) K | b' K ],  Minv = (I + b' stril G)^{-1}
  (U0n = -U0).  Then everything is two tiny serial matmuls per chunk:
  O^T   = Sh0 OcT + O0T,   OcT = Qh^T - Ws^T Fq,  O0T = K^T Fv + U0n^T Fq
  StT'  = Mser^T StT + CcTg,  Mser = g^n (I - Ws^T K),
  CcTg  = g^n (Vnu^T K + K^T U0n),  Fq = triu(K Qh^T), Fv = triu(Vnu Qh^T)
Minv is applied with a truncated Neumann series (I - A)(I + A^2 + A^4 + A^6 + A^8),
A = b' stril(G), realized as R2 = r - A r then 4 Horner steps z <- R2 + A^2 z.
"""

import numpy as np

B, T, C = 4, 1024, 768
NH, HS = 12, 64
HPC = NH // 2            # heads per core
GAMMA, BETA = 0.99, 0.01
BP = BETA / GAMMA        # beta'
NC_ = 128                # chunk length n
NCH = T // NC_           # chunks
GN = GAMMA ** NC_        # gamma^n
NSOLVE = 4               # Horner double-steps

_cache: dict = {}


def _build_program(debug_taps=False):
    import concourse.bass as bass
    import concourse.tile as tile
    from concourse import bacc, mybir

    f32 = mybir.dt.float32
    bf16 = mybir.dt.bfloat16
    Alu = mybir.AluOpType
    Act = mybir.ActivationFunctionType

    nc = bacc.Bacc()

    W3 = 3 * HPC * HS    # 1152
    KT_ = C // 128       # 6 contraction tiles for qkv proj
    KP = HPC * HS // 128  # 3 contraction tiles for out proj

    # ---- DRAM parameters (per-core data; SPMD: same names on all cores) ----
    xT = nc.dram_tensor("xT", [C, T], bf16, kind="ExternalInput")          # x[b].T
    wA = nc.dram_tensor("wA", [C, W3], bf16, kind="ExternalInput")
    wP = nc.dram_tensor("wP", [HPC * HS, C], bf16, kind="ExternalInput")
    xres = nc.dram_tensor("xres", [T, C], bf16, kind="ExternalInput")      # x[b] or 0
    gvec = nc.dram_tensor("gvec", [128, 2], f32, kind="ExternalInput")     # g^(p+1), g^-(p+1)
    masks = nc.dram_tensor("masks", [128, 640], bf16, kind="ExternalInput")
    ign = nc.dram_tensor("ign", [128, 64], bf16, kind="ExternalInput")     # stacked g^n*I64
    ident = nc.dram_tensor("ident", [128, 128], bf16, kind="ExternalInput")
    y = nc.dram_tensor("y", [T, C], f32, kind="ExternalOutput")
    dbg = {}
    if debug_taps:
        for nm, shp, dt_ in [
            ("d_qkv", [128, W3], bf16), ("d_kn", [128, 128], bf16),
            ("d_vn", [128, 128], bf16), ("d_qn", [128, 128], bf16),
            ("d_tsb", [128, 384], bf16), ("d_gu", [128, 128], bf16),
            ("d_gl", [128, 128], bf16), ("d_fq", [128, 128], bf16),
            ("d_pu", [128, 128], bf16), ("d_fv", [128, 128], bf16),
            ("d_rz", [128, 384], bf16), ("d_r2", [128, 384], bf16),
            ("d_n2", [128, 384], bf16), ("d_w", [128, 384], bf16),
            ("d_oc", [128, 128], bf16), ("d_o0", [128, 128], bf16),
            ("d_ms", [128, 64], bf16), ("d_cc", [128, 64], bf16),
            ("d_st", [128, 64], bf16), ("d_ot", [128, 128], bf16),
            ("d_mu", [128, 18], f32), ("d_rstd", [128, 18], f32),
            ("d_bns", [128, 108], f32),
        ]:
            dbg[nm] = nc.dram_tensor(nm, shp, dt_, kind="ExternalOutput")

    with tile.TileContext(nc) as tc:
        with (
            tc.tile_pool(name="persist", bufs=1) as persist,
            tc.tile_pool(name="qkvp", bufs=2) as qkvp,
            tc.tile_pool(name="statp", bufs=2) as statp,
            tc.tile_pool(name="natp", bufs=2) as natp,
            tc.tile_pool(name="tp", bufs=2) as tp,
            tc.tile_pool(name="gramp", bufs=2) as gramp,
            tc.tile_pool(name="solp", bufs=2) as solp,
            tc.tile_pool(name="serp", bufs=3) as serp,
            tc.tile_pool(name="stp", bufs=2) as stp,
            tc.tile_pool(name="yp", bufs=2) as yp,
            tc.tile_pool(name="ps1", bufs=2, space="PSUM") as ps1,
            tc.tile_pool(name="ps2", bufs=2, space="PSUM") as ps2,
            tc.tile_pool(name="ps3", bufs=2, space="PSUM") as ps3,
            tc.tile_pool(name="ps4", bufs=2, space="PSUM") as ps4,
        ):
            # ---- persistent operands ----
            xT_sb = persist.tile([128, KT_, T], bf16)
            for k in range(KT_):
                nc.gpsimd.dma_start(
                    out=xT_sb[:, k, :],
                    in_=xT.rearrange("(k p) t -> p k t", p=128)[:, k, :])
            wA_sb = persist.tile([128, KT_, W3], bf16)
            for k in range(KT_):
                nc.gpsimd.dma_start(
                    out=wA_sb[:, k, :],
                    in_=wA.rearrange("(k p) j -> p k j", p=128)[:, k, :])
            wP_sb = persist.tile([128, KP, C], bf16)
            nc.gpsimd.dma_start(out=wP_sb, in_=wP.rearrange("(k p) j -> p k j", p=128))
            gv_sb = persist.tile([128, 2], f32)
            nc.gpsimd.dma_start(out=gv_sb, in_=gvec[:, :])
            mk_sb = persist.tile([128, 640], bf16)
            nc.gpsimd.dma_start(out=mk_sb, in_=masks[:, :])
            ig_sb = persist.tile([128, 64], bf16)
            nc.gpsimd.dma_start(out=ig_sb, in_=ign[:, :])
            id_sb = persist.tile([128, 128], bf16)
            nc.gpsimd.dma_start(out=id_sb, in_=ident[:, :])
            # O^T accumulator for the whole sequence: [ch=384, t=1024]
            outT_sb = persist.tile([128, KP, T], bf16)

            # masks layout: [0:128 striu*b' | 128:256 triu_incl | 256:384 -striu
            #                | 384:512 triu_incl | 512:640 stril*b']
            M_GU = mk_sb[:, 0:128]
            M_B = mk_sb[:, 0:256]      # [Gu | Fq]
            M_A = mk_sb[:, 256:512]    # [Pu | Fv]
            M_GL = mk_sb[:, 512:640]

            st_cur = []
            for p in range(3):
                stt0 = stp.tile([128, 64], bf16, tag=f"st{p}")
                nc.vector.memset(stt0, 0.0)
                st_cur.append(stt0)

            # per-chunk state passed between pipeline stages
            st_qkv: dict = {}
            st_mach: dict = {}

            st_raw: dict = {}

            def stage_qkv_mm(c):
                t0 = c * NC_
                qkv_sb = qkvp.tile([128, W3], bf16, tag="qkv")
                for nblk in range(3):
                    pp = ps1.tile([128, 384], f32, tag="p1")
                    for k in range(KT_):
                        nc.tensor.matmul(
                            pp[:, :],
                            lhsT=xT_sb[:, k, t0:t0 + 128],
                            rhs=wA_sb[:, k, 384 * nblk:384 * nblk + 384],
                            start=(k == 0), stop=(k == KT_ - 1),
                        )
                    if nblk == 2:
                        nc.scalar.copy(qkv_sb[:, 384 * nblk:384 * nblk + 384],
                                       pp[:, :])
                    else:
                        nc.vector.tensor_copy(
                            qkv_sb[:, 384 * nblk:384 * nblk + 384], pp[:, :])
                st_raw[c] = qkv_sb

            def stage_stats(c):
                qkv_sb = st_raw.pop(c)
                qc = qkv_sb[:, :]
                # ---- stats: per 64-col group mean / unbiased rstd ----
                sq = statp.tile([128, W3], bf16, tag="sq")
                nc.vector.tensor_mul(sq[:, :], qc, qc)
                mu = statp.tile([128, 18], f32, tag="mu")
                nc.vector.tensor_reduce(
                    mu[:, :], qc.rearrange("p (g d) -> p g d", d=64),
                    axis=mybir.AxisListType.X, op=Alu.add)
                m2 = statp.tile([128, 18], f32, tag="m2")
                nc.vector.tensor_reduce(
                    m2[:, :], sq.rearrange("p (g d) -> p g d", d=64),
                    axis=mybir.AxisListType.X, op=Alu.add)
                nc.vector.tensor_scalar_mul(mu[:, :], mu[:, :], 1.0 / 64.0)
                nc.vector.tensor_scalar_mul(m2[:, :], m2[:, :], 1.0 / 64.0)
                v63 = statp.tile([128, 18], f32, tag="v63")
                nc.vector.tensor_mul(v63[:, :], mu[:, :], mu[:, :])
                nc.vector.scalar_tensor_tensor(
                    out=v63[:, :], in0=v63[:, :], scalar=-1.0, in1=m2[:, :],
                    op0=Alu.mult, op1=Alu.add)
                rstd = statp.tile([128, 18], f32, tag="rstd")
                nc.scalar.activation(rstd[:, :], v63[:, :], Act.Sqrt,
                                     scale=64.0 / 63.0)
                nc.vector.reciprocal(rstd[:, :], rstd[:, :])
                # rs18: per-group scale with gamma factors folded in
                rs18 = statp.tile([128, 18], f32, tag="rs18")
                nc.vector.tensor_mul(rs18[:, 0:6], rstd[:, 0:6],
                                     gv_sb[:, 0:1].to_broadcast((128, 6)))
                nc.vector.tensor_copy(rs18[:, 6:12], rstd[:, 6:12])
                nc.vector.tensor_mul(rs18[:, 12:18], rstd[:, 12:18],
                                     gv_sb[:, 1:2].to_broadcast((128, 6)))
                # broadcast to full width (Pool), then two wide DVE ops
                mu_b = statp.tile([128, 18, 64], bf16, tag="mu_b")
                nc.scalar.copy(
                    mu_b[:, :, :],
                    mu.rearrange("p (g o) -> p g o", o=1).to_broadcast((128, 18, 64)))
                rs_b = statp.tile([128, 18, 64], bf16, tag="rs_b")
                nc.scalar.copy(
                    rs_b[:, :, :],
                    rs18.rearrange("p (g o) -> p g o", o=1).to_broadcast((128, 18, 64)))
                qn = qkvp.tile([128, W3], bf16, tag="qn")
                nc.vector.tensor_sub(qn[:, :], qc,
                                     mu_b.rearrange("p g d -> p (g d)"))
                nc.vector.tensor_mul(qn[:, :], qn[:, :],
                                     rs_b.rearrange("p g d -> p (g d)"))
                # kb = -b' * khat into rz triads; u0n filled later by mach
                rz_l = []
                for tri in range(2):
                    rz = solp.tile([128, 384], bf16, tag=f"rz{tri}")
                    nc.scalar.mul(
                        rz[:, :].rearrange("p (j f) -> p j f", f=128)[:, :, 64:128],
                        qn[:, 384 + 192 * tri:576 + 192 * tri].rearrange(
                            "p (j f) -> p j f", f=64), -BP)
                    rz_l.append(rz)
                if debug_taps and c == 0:
                    nc.gpsimd.dma_start(out=dbg["d_qkv"][:, :], in_=qkv_sb[:, :])
                    nc.gpsimd.dma_start(out=dbg["d_mu"][:, :], in_=mu[:, :])
                    nc.gpsimd.dma_start(out=dbg["d_rstd"][:, :], in_=rstd[:, :])
                    nc.gpsimd.dma_start(out=dbg["d_kn"][:, :], in_=qn[:, 384:512])
                    nc.gpsimd.dma_start(out=dbg["d_vn"][:, :], in_=qn[:, 768:896])
                    nc.gpsimd.dma_start(out=dbg["d_qn"][:, :], in_=qn[:, 0:128])
                st_qkv[c] = (qn, rz_l)

            def stage_mach(c):
                qn, rz_l = st_qkv.pop(c)
                knp_l = [qn[:, 384 + 128 * p:512 + 128 * p] for p in range(3)]
                vnp_l = [qn[:, 768 + 128 * p:896 + 128 * p] for p in range(3)]
                qnp_l = [qn[:, 128 * p:128 + 128 * p] for p in range(3)]
                # ---- transposes: tsb = [KTpair | QTpair | VTpair] ----
                tsb_l = []
                for p in range(3):
                    tps = ps4.tile([128, 384], bf16, tag="m4")
                    nc.tensor.transpose(tps[:, 0:128], knp_l[p][:, :], id_sb[:, :])
                    nc.tensor.transpose(tps[:, 128:256], qnp_l[p][:, :], id_sb[:, :])
                    nc.tensor.transpose(tps[:, 256:384], vnp_l[p][:, :], id_sb[:, :])
                    tsb = tp.tile([128, 384], bf16, tag=f"tsb{p}")
                    nc.scalar.copy(tsb[:, :], tps[:, :])
                    tsb_l.append(tsb)
                # ---- grams + masked evacs ----
                gu_l, gl_l, fq_l, pu_l, fv_l = [], [], [], [], []
                for i in range(HPC):
                    p, sub = divmod(i, 2)
                    po = 64 * sub
                    kt = tsb_l[p][po:po + 64, 0:128]
                    vt = tsb_l[p][po:po + 64, 256:384]
                    kq = tsb_l[p][po:po + 64, 0:256]     # [KT | QT]
                    gps = ps2.tile([128, 512], f32, tag="gram")
                    nc.tensor.matmul(gps[:, 0:256], lhsT=kt, rhs=kq,
                                     tile_position=(po, 0), skip_group_check=True)
                    nc.tensor.matmul(gps[:, 256:512], lhsT=vt, rhs=kq,
                                     tile_position=(po, 0), skip_group_check=True)
                    gu = gramp.tile([128, 128], bf16, tag=f"gu{i}")
                    nc.vector.tensor_mul(gu[:, :], gps[:, 0:128], M_GU)
                    fqv = gramp.tile([128, 256], bf16, tag=f"fqv{i}")
                    nc.vector.tensor_mul(
                        fqv[:, :].rearrange("p (a f) -> p a f", f=128),
                        gps[:, :].rearrange("p (a b f) -> p a b f", b=2, f=128)[:, :, 1, :],
                        mk_sb[:, 0:512].rearrange("p (a b f) -> p a b f", b=2, f=128)[:, :, 1, :])
                    pu = gramp.tile([128, 128], bf16, tag=f"pu{i}")
                    nc.scalar.copy(pu[:, :], gps[:, 256:384])
                    nc.gpsimd.affine_select(
                        out=pu[:, :], in_=pu[:, :], compare_op=Alu.is_ge,
                        fill=0.0, base=-1, pattern=[[1, 128]],
                        channel_multiplier=-1)
                    glps = ps4.tile([128, 128], bf16, tag="m4")
                    nc.tensor.transpose(glps[:, :], gu[:, :], id_sb[:, :])
                    gl = gramp.tile([128, 128], bf16, tag=f"gl{i}")
                    if i % 2 == 0:
                        nc.vector.tensor_copy(gl[:, :], glps[:, :])
                    else:
                        nc.scalar.copy(gl[:, :], glps[:, :])
                    gu_l.append(gu[:, 0:128]); fq_l.append(fqv[:, 0:128])
                    pu_l.append(pu[:, 0:128]); fv_l.append(fqv[:, 128:256])
                    gl_l.append(gl)
                if debug_taps and c == 0:
                    nc.gpsimd.dma_start(out=dbg["d_tsb"][:, :], in_=tsb_l[0][:, :])
                    nc.gpsimd.dma_start(out=dbg["d_gu"][:, :], in_=gu_l[0])
                    nc.gpsimd.dma_start(out=dbg["d_gl"][:, :], in_=gl_l[0][:, :])
                    nc.gpsimd.dma_start(out=dbg["d_fq"][:, :], in_=fq_l[0])
                    nc.gpsimd.dma_start(out=dbg["d_pu"][:, :], in_=pu_l[0])
                    nc.gpsimd.dma_start(out=dbg["d_fv"][:, :], in_=fv_l[0])

                # ---- u0n = Pu^T kb  (into rz triad cols 128j:128j+64) ----
                for tri in range(2):
                    ups = ps3.tile([128, 192], f32, tag="sol")
                    for j in range(3):
                        i = 3 * tri + j
                        nc.tensor.matmul(ups[:, 64 * j:64 * j + 64],
                                         lhsT=pu_l[i],
                                         rhs=rz_l[tri][:, 128 * j + 64:128 * j + 128],
                                         skip_group_check=True)
                    nc.scalar.copy(
                        rz_l[tri][:, :].rearrange("p (j f) -> p j f", f=128)[:, :, 0:64],
                        ups[:, :].rearrange("p (j f) -> p j f", f=64))
                # ---- n2 = ((b' stril G)^2)^T ;  n4 = n2 @ n2 via n2T ----
                n2_l, n4_l = [], []
                for tri in range(2):
                    nps = ps3.tile([128, 384], f32, tag="sol")
                    for j in range(3):
                        i = 3 * tri + j
                        nc.tensor.matmul(nps[:, 128 * j:128 * j + 128],
                                         lhsT=gl_l[i][:, :], rhs=gu_l[i],
                                         skip_group_check=True)
                    n2 = solp.tile([128, 384], bf16, tag=f"n2{tri}")
                    if tri == 0:
                        nc.vector.tensor_copy(n2[:, :], nps[:, :])
                    else:
                        nc.scalar.copy(n2[:, :], nps[:, :])
                    n2_l.append(n2)
                for tri in range(2):
                    tps2 = ps4.tile([128, 384], bf16, tag="m4")
                    for j in range(3):
                        nc.tensor.transpose(tps2[:, 128 * j:128 * j + 128],
                                            n2_l[tri][:, 128 * j:128 * j + 128],
                                            id_sb[:, :])
                    n2t = solp.tile([128, 384], bf16, tag=f"n2t{tri}")
                    nc.scalar.copy(n2t[:, :], tps2[:, :])
                    nps4 = ps3.tile([128, 384], f32, tag="sol")
                    for j in range(3):
                        nc.tensor.matmul(nps4[:, 128 * j:128 * j + 128],
                                         lhsT=n2t[:, 128 * j:128 * j + 128],
                                         rhs=n2_l[tri][:, 128 * j:128 * j + 128],
                                         skip_group_check=True)
                    n4 = solp.tile([128, 384], bf16, tag=f"n4{tri}")
                    if tri == 0:
                        nc.vector.tensor_copy(n4[:, :], nps4[:, :])
                    else:
                        nc.scalar.copy(n4[:, :], nps4[:, :])
                    n4_l.append(n4)
                # ---- R2 = (I - A) rz ----
                r2_l = []
                for tri in range(2):
                    rps = ps3.tile([128, 384], f32, tag="sol")
                    nc.tensor.matmul(rps[:, :], lhsT=id_sb[:, :],
                                     rhs=rz_l[tri][:, :],
                                     start=True, stop=False,
                                     skip_group_check=True)
                    for j in range(3):
                        i = 3 * tri + j
                        nc.tensor.matmul(rps[:, 128 * j:128 * j + 128],
                                         lhsT=gu_l[i],
                                         rhs=rz_l[tri][:, 128 * j:128 * j + 128],
                                         start=False, stop=(j == 2),
                                         skip_group_check=True)
                    r2 = solp.tile([128, 384], bf16, tag=f"r2{tri}")
                    if tri == 0:
                        nc.vector.tensor_copy(r2[:, :], rps[:, :])
                    else:
                        nc.scalar.copy(r2[:, :], rps[:, :])
                    r2_l.append(r2)
                # ---- z = (I + A^2)(I + A^4) style: s2 = r2 + A^2 r2; W = s2 + A^4 s2
                z_l = [None, None]
                w_l = [None, None]
                for stage, nn_l in ((0, n2_l), (1, n4_l)):
                    for tri in range(2):
                        src_t = r2_l[tri] if stage == 0 else z_l[tri]
                        zps = ps3.tile([128, 384], f32, tag="sol")
                        nc.tensor.matmul(zps[:, :], lhsT=id_sb[:, :],
                                         rhs=src_t[:, :],
                                         start=True, stop=False,
                                         skip_group_check=True)
                        for j in range(3):
                            nc.tensor.matmul(
                                zps[:, 128 * j:128 * j + 128],
                                lhsT=nn_l[tri][:, 128 * j:128 * j + 128],
                                rhs=src_t[:, 128 * j:128 * j + 128],
                                start=False, stop=(j == 2),
                                skip_group_check=True)
                        tag = f"w{tri}" if stage == 1 else f"z{tri}"
                        zn = solp.tile([128, 384], bf16, tag=tag)
                        if tri == 0:
                            nc.vector.tensor_copy(zn[:, :], zps[:, :])
                        else:
                            nc.scalar.copy(zn[:, :], zps[:, :])
                        if stage == 0:
                            z_l[tri] = zn
                        else:
                            w_l[tri] = zn
                if debug_taps and c == 0:
                    nc.gpsimd.dma_start(out=dbg["d_rz"][:, :], in_=rz_l[0][:, :])
                    nc.gpsimd.dma_start(out=dbg["d_r2"][:, :], in_=r2_l[0][:, :])
                    nc.gpsimd.dma_start(out=dbg["d_n2"][:, :], in_=n2_l[0][:, :])
                    nc.gpsimd.dma_start(out=dbg["d_w"][:, :], in_=w_l[0][:, :])

                def wslice(i, blk):  # blk 0 = U0n, 1 = Ws
                    tri, j = divmod(i, 3)
                    return w_l[tri][:, 128 * j + 64 * blk:128 * j + 64 * blk + 64]

                # ---- OcT = Qh^T - Ws^T Fq ; Mser = g^n (I - Ws^T K) ----
                oc_l, ms_l, o0_l, cc_l = [], [], [], []
                for p in range(3):
                    ocps = ps4.tile([128, 192], f32, tag="m4")
                    nc.tensor.matmul(ocps[:, 0:128], lhsT=id_sb[:, :],
                                     rhs=tsb_l[p][:, 128:256],
                                     start=True, stop=False,
                                     skip_group_check=True)
                    for sub in range(2):
                        i = 2 * p + sub
                        po = 64 * sub
                        nc.tensor.matmul(ocps[po:po + 64, 0:128],
                                         lhsT=wslice(i, 1), rhs=fq_l[i],
                                         tile_position=(0, po),
                                         start=False, stop=(sub == 1),
                                         skip_group_check=True)
                        nc.tensor.matmul(ocps[po:po + 64, 128:192],
                                         lhsT=wslice(i, 1),
                                         rhs=knp_l[p][:, po:po + 64],
                                         tile_position=(0, po),
                                         skip_group_check=True)
                    ocP = serp.tile([128, 128], bf16, tag=f"oc{p}")
                    nc.vector.tensor_copy(ocP[:, :], ocps[:, 0:128])
                    msP = serp.tile([128, 64], bf16, tag=f"ms{p}")
                    nc.vector.scalar_tensor_tensor(
                        out=msP[:, :], in0=ocps[:, 128:192], scalar=GN,
                        in1=ig_sb[:, :], op0=Alu.mult, op1=Alu.add)
                    oc_l.append(ocP); ms_l.append(msP)
                # ---- O0T = K^T Fv + U0n^T Fq ; CcTg = g^n (Vn^T K + K^T U0n) ----
                for p in range(3):
                    o0ps = ps4.tile([128, 192], f32, tag="m4")
                    for sub in range(2):
                        i = 2 * p + sub
                        po = 64 * sub
                        nc.tensor.matmul(o0ps[po:po + 64, 0:128],
                                         lhsT=knp_l[p][:, po:po + 64], rhs=fv_l[i],
                                         tile_position=(0, po), start=True,
                                         stop=False, skip_group_check=True)
                        nc.tensor.matmul(o0ps[po:po + 64, 0:128],
                                         lhsT=wslice(i, 0), rhs=fq_l[i],
                                         tile_position=(0, po), start=False,
                                         stop=True, skip_group_check=True)
                        nc.tensor.matmul(o0ps[po:po + 64, 128:192],
                                         lhsT=vnp_l[p][:, po:po + 64],
                                         rhs=knp_l[p][:, po:po + 64],
                                         tile_position=(0, po), start=True,
                                         stop=False, skip_group_check=True)
                        nc.tensor.matmul(o0ps[po:po + 64, 128:192],
                                         lhsT=knp_l[p][:, po:po + 64],
                                         rhs=wslice(i, 0),
                                         tile_position=(0, po), start=False,
                                         stop=True, skip_group_check=True)
                    o0P = serp.tile([128, 128], bf16, tag=f"o0{p}")
                    nc.scalar.copy(o0P[:, :], o0ps[:, 0:128])
                    ccP = serp.tile([128, 64], bf16, tag=f"cc{p}")
                    nc.scalar.mul(ccP[:, :], o0ps[:, 128:192], GN)
                    o0_l.append(o0P); cc_l.append(ccP)
                if debug_taps and c == 0:
                    nc.gpsimd.dma_start(out=dbg["d_oc"][:, :], in_=oc_l[0][:, :])
                    nc.gpsimd.dma_start(out=dbg["d_o0"][:, :], in_=o0_l[0][:, :])
                    nc.gpsimd.dma_start(out=dbg["d_ms"][:, :], in_=ms_l[0][:, :])
                    nc.gpsimd.dma_start(out=dbg["d_cc"][:, :], in_=cc_l[0][:, :])
                st_mach[c] = (oc_l, ms_l, o0_l, cc_l)

            def stage_ser(c):
                t0 = c * NC_
                oc_l, ms_l, o0_l, cc_l = st_mach.pop(c)
                # ---- O^T chunk = Sh0 @ OcT + O0T ----
                for p in range(3):
                    otps = ps4.tile([128, 128], f32, tag="m4")
                    nc.tensor.matmul(otps[:, :], lhsT=id_sb[:, :],
                                     rhs=o0_l[p][:, :],
                                     start=True, stop=False,
                                     skip_group_check=True)
                    for sub in range(2):
                        po = 64 * sub
                        nc.tensor.matmul(otps[po:po + 64, :],
                                         lhsT=st_cur[p][po:po + 64, :],
                                         rhs=oc_l[p][po:po + 64, :],
                                         tile_position=(po, po),
                                         start=False, stop=(sub == 1),
                                         skip_group_check=True)
                    nc.vector.tensor_copy(outT_sb[:, p, t0:t0 + 128], otps[:, :])
                # ---- state: StT' = Mser^T StT + CcTg ----
                for p in range(3):
                    stps = ps4.tile([128, 64], f32, tag="m4")
                    nc.tensor.matmul(stps[:, :], lhsT=id_sb[:, :],
                                     rhs=cc_l[p][:, :],
                                     start=True, stop=False,
                                     skip_group_check=True)
                    for sub in range(2):
                        po = 64 * sub
                        nc.tensor.matmul(stps[po:po + 64, :],
                                         lhsT=ms_l[p][po:po + 64, :],
                                         rhs=st_cur[p][po:po + 64, :],
                                         tile_position=(po, po),
                                         start=False, stop=(sub == 1),
                                         skip_group_check=True)
                    stn = stp.tile([128, 64], bf16, tag=f"st{p}")
                    nc.scalar.copy(stn[:, :], stps[:, :])
                    st_cur[p] = stn
                if debug_taps and c == 0:
                    nc.gpsimd.dma_start(out=dbg["d_st"][:, :], in_=st_cur[0][:, :])
                    nc.gpsimd.dma_start(out=dbg["d_ot"][:, :],
                                        in_=outT_sb[:, 0, t0:t0 + 128])
                # ---- output projection rows t0:t0+128 + residual ----
                xr_sb = yp.tile([128, C], bf16, tag="xr")
                nc.gpsimd.dma_start(out=xr_sb[:, :], in_=xres[t0:t0 + 128, :])
                y_sb = yp.tile([128, C], f32, tag="ysb")
                for nblk in range(2):
                    ypp = ps1.tile([128, 384], f32, tag="p1")
                    nc.tensor.matmul(ypp[:, :], lhsT=id_sb[:, :],
                                     rhs=xr_sb[:, 384 * nblk:384 * nblk + 384],
                                     start=True, stop=False,
                                     skip_group_check=True)
                    for k in range(KP):
                        nc.tensor.matmul(
                            ypp[:, :],
                            lhsT=outT_sb[:, k, t0:t0 + 128],
                            rhs=wP_sb[:, k, 384 * nblk:384 * nblk + 384],
                            start=False, stop=(k == KP - 1),
                            skip_group_check=True,
                        )
                    nc.scalar.copy(y_sb[:, 384 * nblk:384 * nblk + 384],
                                   ypp[:, :])
                nc.gpsimd.dma_start(out=y[t0:t0 + 128, :], in_=y_sb[:, :])

            # ---- software-pipelined emission ----
            stage_qkv_mm(0)
            stage_stats(0)
            stage_qkv_mm(1)
            stage_stats(1)
            stage_mach(0)
            for c in range(2, NCH):
                stage_qkv_mm(c)
                stage_mach(c - 1)
                stage_ser(c - 2)
                stage_stats(c)
            stage_mach(NCH - 1)
            stage_ser(NCH - 2)
            stage_ser(NCH - 1)

    nc.finalize()
    return nc


def _host_inputs(x, w_attn, w_proj):
    """Build the 8 per-core input maps."""
    import ml_dtypes
    bf = ml_dtypes.bfloat16
    in_maps = []
    gvec = np.zeros((128, 2), np.float32)
    p = np.arange(1, 129, dtype=np.float64)
    gvec[:, 0] = GAMMA ** p
    gvec[:, 1] = GAMMA ** (-p)
    striu = np.triu(np.ones((128, 128), np.float32), 1)
    triui = np.triu(np.ones((128, 128), np.float32))
    stril = np.tril(np.ones((128, 128), np.float32), -1)
    masks = np.concatenate(
        [-striu * BP, triui, striu, triui, -stril * BP], axis=1).astype(bf)
    ign = np.concatenate([np.eye(64), np.eye(64)], axis=0).astype(np.float32)
    ign = (ign * GN).astype(bf)
    ident = np.eye(128, dtype=np.float32).astype(bf)
    for core in range(8):
        b, hg = divmod(core, 2)
        h0 = hg * HPC
        cols = []
        for blk in range(3):   # q, k, v column blocks of w_attn
            cols.append(w_attn[:, blk * C + h0 * HS: blk * C + (h0 + HPC) * HS])
        wA_s = np.ascontiguousarray(np.concatenate(cols, axis=1)).astype(bf)
        wP_s = np.ascontiguousarray(w_proj[h0 * HS:(h0 + HPC) * HS]).astype(bf)
        xb = np.ascontiguousarray(x[b])                                # [1024, 768]
        xres = xb.astype(bf) if hg == 0 else np.zeros((T, C), bf)
        in_maps.append({
            "xT": np.ascontiguousarray(xb.T).astype(bf),
            "wA": wA_s,
            "wP": wP_s,
            "xres": xres,
            "gvec": gvec,
            "masks": masks,
            "ign": ign,
            "ident": ident,
        })
    return in_maps


def kernel(x, w_attn, w_proj):
    from concourse.bass_utils import run_bass_kernel_spmd

    if "nc" not in _cache:
        _cache["nc"] = _build_program()
    nc = _cache["nc"]

    in_maps = _host_inputs(np.asarray(x), np.asarray(w_attn), np.asarray(w_proj))
    res = run_bass_kernel_spmd(nc, in_maps, core_ids=list(range(8)))
    out = np.empty((B, T, C), np.float32)
    for b in range(B):
        out[b] = res.results[2 * b]["y"] + res.results[2 * b + 1]["y"]
    return out
